# revision 52
# baseline (speedup 1.0000x reference)
"""Trainium2 Bass kernel for nn_EnergyAdaptor (segment_reduce).

Data-parallel over batch: 32 batches -> 8 cores x 4 batches.
Main path in [CH, T] layout; convs as PE matmuls (float32r).
Segment reduce via per-group window gather (indirect DMA fetches one
32-element window per partition) + masked reductions on DVE.
"""

import functools

import numpy as np

import concourse.bass as bass
import concourse.mybir as mybir
import concourse.tile as tile
from concourse.bass_utils import run_bass_kernel_spmd

dt = mybir.dt
AF = mybir.ActivationFunctionType
ALU = mybir.AluOpType

B, T, CIN, CH, TDE = 32, 1024, 512, 256, 4096
K1, KE = 5, 3
NCORES = 8
BPC = B // NCORES  # batches per core
LEAKY, EPS = 0.3, 1e-5
WIN = 32  # window width: max span of 8 segments = 24 < 32

PROFILE = False
LAST_EXEC_NS = None
LAST_RESULTS = None

F32 = dt.float32
F32R = dt.float32r
I32 = dt.int32


def _mm(nc, out, lhsT, rhs, start, stop):
    """fast matmul: operands must be float32r-typed APs."""
    assert lhsT.dtype == F32R and rhs.dtype == F32R, (lhsT.dtype, rhs.dtype)
    nc.tensor.matmul(out, lhsT, rhs, start=start, stop=stop)


def _split_multi_waits(nc, max_waits=1):
    """TRN2 codegen workaround: most instruction encodings accept at most
    one embedded sem wait; hoist extras onto preceding same-engine NoOps."""
    n = 0
    for bb in nc.main_func.blocks:
        out = []
        for ins in bb.instructions:
            si = ins.sync_info
            waits = list(si.on_wait) if si is not None and si.on_wait else []
            limit = 0 if isinstance(ins, mybir.InstDrain) else max_waits
            if len(waits) > limit:
                extras, keep = waits[: len(waits) - limit], waits[len(waits) - limit:]
                for w in extras:
                    out.append(
                        mybir.InstNoOp(
                            name=f"{ins.name}-wsp{n}", engine=ins.engine,
                            ins=[], outs=[],
                            sync_info=mybir.SyncInfo(on_wait=[w], on_update=[]),
                        )
                    )
                    n += 1
                ins.sync_info = mybir.SyncInfo(
                    on_wait=keep, on_update=list(si.on_update)
                )
            out.append(ins)
        bb.instructions = out
    return n


def _build():
    nc = bass.Bass()

    x_d = nc.dram_tensor("x", [BPC, CIN, T], F32R, kind="ExternalInput")
    tgt_d = nc.dram_tensor("tgt", [BPC * TDE], F32, kind="ExternalInput")
    dr_d = nc.dram_tensor("drf", [BPC, T], F32, kind="ExternalInput")
    w1_d = nc.dram_tensor("w1", [CIN, K1 * CH], F32R, kind="ExternalInput")
    w2_d = nc.dram_tensor("w2", [CH, K1 * CH], F32R, kind="ExternalInput")
    wlin_d = nc.dram_tensor("wlin", [CH], F32R, kind="ExternalInput")
    wlinS_d = nc.dram_tensor("wlinS", [1], F32R, kind="ExternalInput")
    blin_d = nc.dram_tensor("blin", [1], F32, kind="ExternalInput")
    b1_d = nc.dram_tensor("b1", [CH], F32, kind="ExternalInput")
    b2_d = nc.dram_tensor("b2", [CH], F32, kind="ExternalInput")
    bemb_d = nc.dram_tensor("bemb", [CH], F32, kind="ExternalInput")
    wemb_d = nc.dram_tensor("wemb", [KE, CH], F32R, kind="ExternalInput")
    iden_d = nc.dram_tensor("iden", [128, 128], F32, kind="ExternalInput")
    tri_d = nc.dram_tensor("tri", [128, 128], F32R, kind="ExternalInput")
    triu1_d = nc.dram_tensor("triu1", [128, 128], F32, kind="ExternalInput")
    jramp_d = nc.dram_tensor("jramp", [128, 512], F32, kind="ExternalInput")

    pred_d = nc.dram_tensor("pred", [BPC, T], F32, kind="ExternalOutput")
    avg_d = nc.dram_tensor("avg", [BPC, T], F32, kind="ExternalOutput")
    emb_d = nc.dram_tensor("emb", [BPC, CH, T], F32, kind="ExternalOutput")

    with tile.TileContext(nc) as tc:
        with (
            tc.tile_pool(name="const", bufs=1) as cpool,
            tc.tile_pool(name="work", bufs=2) as wpool,
            tc.tile_pool(name="seg", bufs=2) as spool,
            tc.tile_pool(name="rows", bufs=2) as rpool,
            tc.tile_pool(name="ps_conv", bufs=2, space="PSUM") as pconv,
            tc.tile_pool(name="ps_emb", bufs=1, space="PSUM") as pemb,
            tc.tile_pool(name="ps_reps", bufs=2, space="PSUM") as preps,
            tc.tile_pool(name="ps_stat", bufs=3, space="PSUM") as pstat,
        ):
            # ---- persistent constants / params ----
            iden = cpool.tile([128, 128], F32, tag="iden")
            tri = cpool.tile([128, 128], F32R, tag="tri")
            triu1 = cpool.tile([128, 128], F32, tag="triu1")
            jramp = cpool.tile([128, 512], F32, tag="jramp")
            w1sb = cpool.tile([128, 4 * K1 * CH], F32R, tag="w1")
            w2sb = cpool.tile([128, 2 * K1 * CH], F32R, tag="w2")
            wlin = cpool.tile([128, 2], F32R, tag="wlin")
            wlinS = cpool.tile([1, 1], F32R, tag="wlinS")
            blin = cpool.tile([1, 1], F32, tag="blin")
            b1c = cpool.tile([128, 2], F32, tag="b1c")
            b2c = cpool.tile([128, 2], F32, tag="b2c")
            bembc = cpool.tile([128, 2], F32, tag="bembc")
            wembT = cpool.tile([KE, CH], F32R, tag="wembT")
            epsc = cpool.tile([128, 1], F32, tag="epsc")
            zedge = cpool.tile([128, 8], F32, tag="zedge")

            nc.scalar.dma_start(iden[:], iden_d[:])
            nc.scalar.dma_start(tri[:], tri_d[:])
            nc.scalar.dma_start(triu1[:], triu1_d[:])
            nc.scalar.dma_start(jramp[:], jramp_d[:])
            for c in range(4):
                eng = [nc.sync, nc.scalar, nc.gpsimd, nc.sync][c]
                eng.dma_start(
                    w1sb[:, c * K1 * CH : (c + 1) * K1 * CH],
                    w1_d[c * 128 : (c + 1) * 128, :],
                )
            for c in range(2):
                eng = [nc.gpsimd, nc.scalar][c]
                eng.dma_start(
                    w2sb[:, c * K1 * CH : (c + 1) * K1 * CH],
                    w2_d[c * 128 : (c + 1) * 128, :],
                )
            for c in range(2):
                sl = slice(c * 128, (c + 1) * 128)
                nc.gpsimd.dma_start(
                    wlin[:, c : c + 1], wlin_d[sl].rearrange("(a b) -> a b", b=1)
                )
                nc.gpsimd.dma_start(
                    b1c[:, c : c + 1], b1_d[sl].rearrange("(a b) -> a b", b=1)
                )
                nc.gpsimd.dma_start(
                    b2c[:, c : c + 1], b2_d[sl].rearrange("(a b) -> a b", b=1)
                )
                nc.gpsimd.dma_start(
                    bembc[:, c : c + 1], bemb_d[sl].rearrange("(a b) -> a b", b=1)
                )
            nc.gpsimd.dma_start(blin[:], blin_d[:].rearrange("(a b) -> a b", b=1))
            nc.gpsimd.dma_start(wlinS[:], wlinS_d[:].rearrange("(a b) -> a b", b=1))
            nc.gpsimd.dma_start(wembT[:], wemb_d[:])
            nc.gpsimd.memset(epsc[:], EPS)
            nc.gpsimd.memset(zedge[:], 0.0)

            ones_row = tri[0:1, :]  # [1,128] all ones (f32r)
            ones_col = tri[:, 127:128]  # [128,1] all ones (f32r)

            def seg(b):
                _emit_segment(
                    nc, tc, b, tgt_d, dr_d, avg_d, emb_d,
                    iden, triu1, jramp, wembT, bembc,
                    wpool, spool, pemb, pstat,
                )

            def main(b):
                _emit_main(
                    nc, tc, b, x_d, pred_d,
                    iden, tri, ones_row, ones_col,
                    w1sb, w2sb, wlin, wlinS, blin, b1c, b2c, epsc, zedge,
                    wpool, rpool, pconv, preps, pstat, first=(b < 2),
                )

            main(0); seg(0); seg(1); main(1); seg(2); main(2); seg(3); main(3)

    _split_multi_waits(nc)
    return nc


def _emit_segment(
    nc, tc, b, tgt_d, dr_d, avg_d, emb_d,
    iden, triu1, jramp, wembT, bembc,
    wpool, spool, paux, pstat,
):
    """avg_energy_target for batch b + energy_emb conv.

    Row-major grouping: t = g*8 + i with group g on partition g.
    """
    # ends = inclusive cumsum of dr, via free-dim Hillis-Steele within each
    # 8-wide group plus a partition-dim exclusive prefix of group totals.
    dr_rm = spool.tile([128, 8], F32, tag="dr_rm")
    nc.sync.dma_start(dr_rm[:], dr_d[b, :].rearrange("(a b) -> a b", b=8))
    p1 = spool.tile([128, 8], F32, tag="p1")
    nc.vector.tensor_copy(p1[:, 0:1], dr_rm[:, 0:1])
    nc.vector.tensor_tensor(out=p1[:, 1:8], in0=dr_rm[:, 1:8], in1=dr_rm[:, 0:7], op=ALU.add)
    p2 = spool.tile([128, 8], F32, tag="p2")
    nc.vector.tensor_copy(p2[:, 0:2], p1[:, 0:2])
    nc.vector.tensor_tensor(out=p2[:, 2:8], in0=p1[:, 2:8], in1=p1[:, 0:6], op=ALU.add)
    ends = spool.tile([128, 8], F32, tag="ends")
    nc.vector.tensor_copy(ends[:, 0:4], p2[:, 0:4])
    nc.vector.tensor_tensor(out=ends[:, 4:8], in0=p2[:, 4:8], in1=p2[:, 0:4], op=ALU.add)

    gt_ps = pstat.tile([128, 512], F32, tag="stat")
    nc.tensor.matmul(gt_ps[:, 0:1], triu1[:], ends[:, 7:8], start=True, stop=True)
    offc = spool.tile([128, 1], F32, tag="offc")
    nc.scalar.activation(offc[:], gt_ps[:, 0:1], AF.Identity)
    nc.vector.tensor_scalar_add(ends[:], ends[:], offc[:, 0:1])
    starts = spool.tile([128, 8], F32, tag="starts")
    nc.vector.tensor_tensor(out=starts[:], in0=ends[:], in1=dr_rm[:], op=ALU.subtract)

    # window gather: one offset per partition (group), WIN consecutive values
    off_f = spool.tile([128, 1], F32, tag="off_f")
    nc.vector.tensor_scalar_add(off_f[:], starts[:, 0:1], float(b * TDE))
    off_i = spool.tile([128, 1], I32, tag="off_i")
    nc.vector.tensor_copy(off_i[:], off_f[:])
    W = spool.tile([128, WIN], F32, tag="W")
    nc.gpsimd.indirect_dma_start(
        out=W[:], out_offset=None,
        in_=tgt_d[:].rearrange("(a b) -> a b", b=1),
        in_offset=bass.IndirectOffsetOnAxis(ap=off_i[:], axis=0),
    )
    wnz = spool.tile([128, 2 * WIN], F32, tag="wnz")
    nc.vector.tensor_copy(wnz[:, 0:WIN], W[:])
    nc.vector.tensor_scalar(wnz[:, WIN : 2 * WIN], W[:], 0.0, None, ALU.not_equal)

    # per-partition relative end positions, then masked prefix reductions
    erel = spool.tile([128, 8], F32, tag="erel")
    nc.vector.tensor_scalar(
        erel[:], ends[:], starts[:, 0:1], None, ALU.subtract
    )
    # sene[:, 2i] = prefix sum_i, sene[:, 2i+1] = prefix count_i
    bigmask = spool.tile([128, 512], F32, tag="bigmask")
    erel_b = erel[:].rearrange("p (i o) -> p i o", o=1).to_broadcast((128, 8, 2 * WIN))
    nc.vector.tensor_tensor(
        out=bigmask[:].rearrange("p (i w) -> p i w", i=8),
        in0=jramp[:].rearrange("p (i w) -> p i w", i=8),
        in1=erel_b, op=ALU.is_lt,
    )
    bigprod = spool.tile([128, 512], F32, tag="bigprod")
    wnz_b = wnz[:].rearrange("p (o w) -> p o w", o=1).to_broadcast((128, 8, 2 * WIN))
    nc.vector.tensor_tensor(
        out=bigprod[:].rearrange("p (i w) -> p i w", i=8),
        in0=bigmask[:].rearrange("p (i w) -> p i w", i=8),
        in1=wnz_b, op=ALU.mult,
    )
    sene = spool.tile([128, 16], F32, tag="sene")
    nc.vector.tensor_reduce(
        out=sene[:].rearrange("p (i g) -> p i g", g=2),
        in_=bigprod[:].rearrange("p (i g w) -> p i g w", i=8, g=2),
        axis=mybir.AxisListType.X, op=ALU.add,
    )
    sene_r = sene[:].rearrange("p (i g) -> p i g", g=2)

    # segment sums / counts = adjacent differences of the prefix values
    sums = spool.tile([128, 8], F32, tag="sums")
    nc.vector.tensor_copy(sums[:, 0:1], sene[:, 0:1])
    nc.vector.tensor_tensor(
        out=sums[:, 1:8].rearrange("p (i o) -> p i o", o=1),
        in0=sene_r[:, 1:8, 0:1], in1=sene_r[:, 0:7, 0:1], op=ALU.subtract)
    nel = spool.tile([128, 8], F32, tag="nel")
    nc.vector.tensor_copy(nel[:, 0:1], sene[:, 1:2])
    nc.vector.tensor_tensor(
        out=nel[:, 1:8].rearrange("p (i o) -> p i o", o=1),
        in0=sene_r[:, 1:8, 1:2], in1=sene_r[:, 0:7, 1:2], op=ALU.subtract)

    dmax = spool.tile([128, 8], F32, tag="dmax")
    nc.vector.tensor_scalar_max(dmax[:], nel[:], 1.0)
    rec = spool.tile([128, 8], F32, tag="rec")
    nc.vector.reciprocal(rec[:], dmax[:])
    ind = spool.tile([128, 8], F32, tag="ind")
    nc.vector.tensor_scalar_min(ind[:], nel[:], 1.0)
    avg = spool.tile([128, 8], F32, tag="avgt")
    nc.vector.tensor_tensor(out=avg[:], in0=sums[:], in1=rec[:], op=ALU.mult)
    nc.vector.tensor_tensor(out=avg[:], in0=avg[:], in1=ind[:], op=ALU.mult)
    nc.gpsimd.dma_start(avg_d[b, :].rearrange("(a b) -> a b", b=8), avg[:])

    # --- energy_emb: K=3 matmul; rows built via SBUF shift DMAs ---
    row3s = spool.tile([KE, 1028], F32, tag="row3s")
    nc.gpsimd.memset(row3s[:], 0.0)
    nc.sync.dma_start(row3s[1:2, 0:1024], avg[:])
    nc.sync.dma_start(row3s[0:1, 1:1025], row3s[1:2, 0:1024])
    nc.sync.dma_start(row3s[2:3, 0:1023], row3s[1:2, 1:1024])
    row3 = spool.tile([KE, 1028], F32R, tag="row3")
    nc.vector.tensor_copy(row3[:], row3s[:])
    for cc in range(2):
        for tt0 in (0, 512):
            emb_ps = paux.tile([128, 512], F32, tag="embp")
            _mm(
                nc, emb_ps[:],
                wembT[:, cc * 128 : (cc + 1) * 128],
                row3[:, tt0 : tt0 + 512],
                start=True, stop=True,
            )
            emb_sb = wpool.tile([128, 512], F32, tag="emb_sb")
            nc.scalar.activation(
                emb_sb[:], emb_ps[:], AF.Identity, bias=bembc[:, cc : cc + 1]
            )
            nc.gpsimd.dma_start(
                emb_d[b, cc * 128 : (cc + 1) * 128, tt0 : tt0 + 512], emb_sb[:]
            )


def _emit_main(
    nc, tc, b, x_d, pred_d,
    iden, tri, ones_row, ones_col,
    w1sb, w2sb, wlin, wlinS, blin, b1c, b2c, epsc, zedge,
    wpool, rpool, pconv, preps, pstat, first=True,
):
    """VariancePredictor for batch b -> pred_d[b]."""
    # --- load + transpose x[b] to [CIN, T] padded ---
    xT = [wpool.tile([128, 1028], F32R, tag=f"xT{c}", name=f"xT{c}") for c in range(4)]
    if first:
        for c in range(4):
            nc.scalar.activation(xT[c][:, 0:2], zedge[:, 0:2], AF.Identity)
            nc.scalar.activation(xT[c][:, 1026:1028], zedge[:, 0:2], AF.Identity)
    for half in range(2):
        for c in range(4):
            nc.sync.dma_start(
                xT[c][:, 2 + half * 512 : 2 + (half + 1) * 512],
                x_d[b, c * 128 : (c + 1) * 128, half * 512 : (half + 1) * 512],
            )

    # --- conv1 + lrelu -> h1 (padded) ---
    h1 = [wpool.tile([128, 1028], F32R, tag=f"h1_{cc}", name=f"h1_{cc}") for cc in range(2)]
    if first:
        for cc in range(2):
            nc.scalar.activation(h1[cc][:, 0:2], zedge[:, 0:2], AF.Identity)
            nc.scalar.activation(h1[cc][:, 1026:1028], zedge[:, 0:2], AF.Identity)
    for tt0 in (0, 512):
        for cc in range(2):
            cps = pconv.tile([128, 512], F32, tag="conv")
            n = 0
            for ci in range(4):
                for k in range(K1):
                    _mm(
                        nc, cps[:],
                        w1sb[:, (ci * K1 + k) * CH + cc * 128 : (ci * K1 + k) * CH + cc * 128 + 128],
                        xT[ci][:, tt0 + k : tt0 + k + 512],
                        start=(n == 0), stop=(n == 19),
                    )
                    n += 1
            nc.scalar.activation(
                h1[cc][:, 2 + tt0 : 2 + tt0 + 512], cps[:], AF.Prelu,
                bias=b1c[:, cc : cc + 1], alpha=LEAKY,
            )

    # --- LN1 (normalize only; affine folded into w2/b2) ---
    _emit_ln(nc, b, 0, h1, 2, tri, ones_row, ones_col, epsc, wpool, rpool, preps, pstat)

    # --- conv2 + lrelu -> h2 ---
    h2 = [wpool.tile([128, 1024], F32R, tag=f"h2_{cc}", name=f"h2_{cc}") for cc in range(2)]
    for tt0 in (0, 512):
        for cc in range(2):
            cps = pconv.tile([128, 512], F32, tag="conv")
            n = 0
            for ci in range(2):
                for k in range(K1):
                    _mm(
                        nc, cps[:],
                        w2sb[:, (ci * K1 + k) * CH + cc * 128 : (ci * K1 + k) * CH + cc * 128 + 128],
                        h1[ci][:, tt0 + k : tt0 + k + 512],
                        start=(n == 0), stop=(n == 9),
                    )
                    n += 1
            nc.scalar.activation(
                h2[cc][:, tt0 : tt0 + 512], cps[:], AF.Prelu,
                bias=b2c[:, cc : cc + 1], alpha=LEAKY,
            )

    # --- LN2 (shift folded into pred matmul) ---
    wf2 = rpool.tile([1, 1024], F32R, tag="wf2")
    _emit_ln(nc, b, 1, h2, 0, tri, ones_row, ones_col, epsc, wpool, rpool, preps, pstat, w_out=wf2)

    # --- linear -> pred (+ S * w term) ---
    pred_sb = rpool.tile([1, 1024], F32, tag="pred_sb")
    for tt0 in (0, 512):
        pps = pstat.tile([1, 512], F32, tag="stat")
        _mm(nc, pps[:], wlin[:, 0:1], h2[0][:, tt0 : tt0 + 512], start=True, stop=False)
        _mm(nc, pps[:], wlin[:, 1:2], h2[1][:, tt0 : tt0 + 512], start=False, stop=False)
        _mm(nc, pps[:], wlinS[0:1, 0:1], wf2[0:1, tt0 : tt0 + 512], start=False, stop=True)
        nc.scalar.activation(
            pred_sb[:, tt0 : tt0 + 512], pps[:], AF.Identity, bias=blin[0:1, 0:1]
        )
    nc.gpsimd.dma_start(pred_d[b, :].rearrange("(a b) -> a b", a=1), pred_sb[:])


def _emit_ln(nc, b, which, h, pad, tri, ones_row, ones_col, epsc, wpool, rpool, preps, pstat, w_out=None):
    """In-place layernorm (normalize only) over channel dim of h ([CH,T] layout)."""
    inv_c = 1.0 / CH
    for tt0 in (0, 512):
        sl = slice(pad + tt0, pad + tt0 + 512)
        s1t = pstat.tile([1, 512], F32, tag="stat")
        s1 = s1t[:]
        _mm(nc, s1, ones_col, h[0][:, sl], start=True, stop=False)
        _mm(nc, s1, ones_col, h[1][:, sl], start=False, stop=True)
        negm = rpool.tile([1, 512], F32, tag="negm")
        nc.scalar.activation(negm[:], s1, AF.Identity, scale=-inv_c)
        s2t = pstat.tile([1, 512], F32, tag="stat")
        s2 = s2t[:]
        for cc in range(2):
            sq = wpool.tile([128, 512], F32R, tag="sq")
            nc.scalar.activation(sq[:], h[cc][:, sl], AF.Square)
            _mm(nc, s2, ones_col, sq[:], start=(cc == 0), stop=(cc == 1))
        msq = rpool.tile([1, 512], F32, tag="msq")
        nc.vector.tensor_tensor(out=msq[:], in0=negm[:], in1=negm[:], op=ALU.mult)
        s2c = rpool.tile([1, 512], F32, tag="s2c")
        nc.scalar.activation(s2c[:], s2, AF.Identity, scale=inv_c)
        var = rpool.tile([1, 512], F32, tag="var")
        nc.vector.tensor_tensor(out=var[:], in0=s2c[:], in1=msq[:], op=ALU.subtract)
        sd = rpool.tile([1, 512], F32, tag="sd")
        nc.scalar.activation(sd[:], var[:], AF.Sqrt, bias=epsc[0:1, 0:1])
        a_row = rpool.tile([1, 512], F32R, tag="a_row")
        with nc.allow_low_precision(reason="f32r operand for PE replicate"):
            nc.vector.reciprocal(a_row[:], sd[:])
        if w_out is not None:
            # shift folded downstream: h <- a * h only; w written to w_out row
            nc.vector.tensor_tensor(
                out=w_out[0:1, pad + tt0 : pad + tt0 + 512],
                in0=negm[:], in1=a_row[:].bitcast(F32), op=ALU.mult,
            )
            a_rep = preps.tile([128, 512], F32, tag="reps")
            _mm(nc, a_rep[:], ones_row, a_row[:], start=True, stop=True)
            for cc in range(2):
                nc.vector.tensor_tensor(
                    out=h[cc][:, sl], in0=h[cc][:, sl], in1=a_rep[:], op=ALU.mult
                )
        else:
            w_row = rpool.tile([1, 512], F32R, tag="w_row")
            nc.vector.tensor_tensor(out=w_row[:], in0=negm[:], in1=a_row[:].bitcast(F32), op=ALU.mult)
            a_rep = preps.tile([128, 512], F32, tag="reps")
            _mm(nc, a_rep[:], ones_row, a_row[:], start=True, stop=True)
            w_rep = preps.tile([128, 512], F32, tag="reps")
            _mm(nc, w_rep[:], ones_row, w_row[:], start=True, stop=True)
            for cc in range(2):
                nc.vector.tensor_tensor(
                    out=h[cc][:, sl], in0=h[cc][:, sl], in1=a_rep[:], op=ALU.mult
                )
                nc.vector.tensor_tensor(
                    out=h[cc][:, sl], in0=h[cc][:, sl], in1=w_rep[:], op=ALU.add
                )


@functools.lru_cache(maxsize=1)
def _get_nc():
    return _build()


def kernel(
    x, target, dr, mask,
    w_conv1, b_conv1, ln1_g, ln1_b,
    w_conv2, b_conv2, ln2_g, ln2_b,
    w_lin, b_lin, w_emb, b_emb,
):
    global LAST_EXEC_NS, LAST_RESULTS

    x = np.ascontiguousarray(np.asarray(x, np.float32).transpose(0, 2, 1))
    tgt = np.ascontiguousarray(np.asarray(target, np.float32).reshape(B, TDE))
    drf = np.ascontiguousarray(np.asarray(dr).astype(np.float32))
    mask_np = np.asarray(mask)

    w1h = np.asarray(w_conv1, np.float32)  # [CH, CIN, K]
    g1 = np.asarray(ln1_g, np.float32)
    b1ln = np.asarray(ln1_b, np.float32)
    w2h = np.asarray(w_conv2, np.float32) * g1[None, :, None]
    b2_eff = np.asarray(b_conv2, np.float32) + np.einsum(
        "oik,i->o", np.asarray(w_conv2, np.float32), b1ln
    )
    g2 = np.asarray(ln2_g, np.float32)
    b2ln = np.asarray(ln2_b, np.float32)
    wlin_h = np.asarray(w_lin, np.float32)[:, 0]
    wlin_eff = wlin_h * g2
    blin_eff = (np.asarray(b_lin, np.float32) + wlin_h @ b2ln).reshape(1)
    wlinS_v = np.asarray([wlin_eff.sum()], np.float32)

    w1 = np.ascontiguousarray(w1h.transpose(1, 2, 0).reshape(CIN, K1 * CH))
    w2 = np.ascontiguousarray(w2h.transpose(1, 2, 0).reshape(CH, K1 * CH))
    wembT = np.ascontiguousarray(np.asarray(w_emb, np.float32)[:, 0, :].T)  # [KE, CH]
    b1c = np.asarray(b_conv1, np.float32)
    bemb = np.asarray(b_emb, np.float32)

    iden = np.eye(128, dtype=np.float32)
    tri = np.triu(np.ones((128, 128), np.float32))
    triu1 = np.triu(np.ones((128, 128), np.float32), 1)
    jr = np.tile(np.concatenate([np.arange(WIN), np.arange(WIN)]), 8).astype(np.float32)
    jramp = np.broadcast_to(jr[None, :], (128, 512)).copy()

    shared = {
        "w1": w1, "w2": w2, "wlin": np.ascontiguousarray(wlin_eff),
        "blin": np.ascontiguousarray(blin_eff),
        "wlinS": wlinS_v,
        "b1": np.ascontiguousarray(b1c), "b2": np.ascontiguousarray(b2_eff),
        "bemb": np.ascontiguousarray(bemb), "wemb": wembT,
        "iden": iden, "tri": tri, "triu1": triu1, "jramp": jramp,
    }
    in_maps = []
    for c in range(NCORES):
        sl = slice(c * BPC, (c + 1) * BPC)
        in_maps.append(
            {
                "x": np.ascontiguousarray(x[sl]),
                "tgt": np.ascontiguousarray(tgt[sl]).reshape(-1),
                "drf": np.ascontiguousarray(drf[sl]),
                **shared,
            }
        )

    nc = _get_nc()
    res = run_bass_kernel_spmd(nc, in_maps, list(range(NCORES)), trace=PROFILE)
    LAST_EXEC_NS = res.exec_time_ns
    LAST_RESULTS = res

    pred = np.concatenate([r["pred"] for r in res.results], axis=0)
    avg = np.concatenate([r["avg"] for r in res.results], axis=0)
    emb = np.concatenate([r["emb"] for r in res.results], axis=0)

    pred = np.where(mask_np, np.float32(0.0), pred).astype(np.float32)
    return pred, avg.reshape(B, 1, T), emb


# revision 62
# speedup vs baseline: 1.0306x; 1.0306x over previous
"""Trainium2 Bass kernel for nn_EnergyAdaptor (segment_reduce).

Data-parallel over batch: 32 batches -> 8 cores x 4 batches.
Main path in [CH, T] layout; convs as PE matmuls (float32r).
Segment reduce via per-group window gather (indirect DMA fetches one
32-element window per partition) + masked reductions on DVE.
"""

import functools

import numpy as np

import concourse.bass as bass
import concourse.mybir as mybir
import concourse.tile as tile
from concourse.bass_utils import run_bass_kernel_spmd

dt = mybir.dt
AF = mybir.ActivationFunctionType
ALU = mybir.AluOpType

B, T, CIN, CH, TDE = 32, 1024, 512, 256, 4096
K1, KE = 5, 3
NCORES = 8
BPC = B // NCORES  # batches per core
LEAKY, EPS = 0.3, 1e-5
WIN = 32  # window width: max span of 8 segments = 24 < 32

PROFILE = False
LAST_EXEC_NS = None
LAST_RESULTS = None

F32 = dt.float32
F32R = dt.float32r
I32 = dt.int32


def _mm(nc, out, lhsT, rhs, start, stop):
    """fast matmul: operands must be float32r-typed APs."""
    assert lhsT.dtype == F32R and rhs.dtype == F32R, (lhsT.dtype, rhs.dtype)
    nc.tensor.matmul(out, lhsT, rhs, start=start, stop=stop)


def _split_multi_waits(nc, max_waits=1):
    """TRN2 codegen workaround: most instruction encodings accept at most
    one embedded sem wait; hoist extras onto preceding same-engine NoOps."""
    n = 0
    for bb in nc.main_func.blocks:
        out = []
        for ins in bb.instructions:
            si = ins.sync_info
            waits = list(si.on_wait) if si is not None and si.on_wait else []
            limit = 0 if isinstance(ins, mybir.InstDrain) else max_waits
            if len(waits) > limit:
                extras, keep = waits[: len(waits) - limit], waits[len(waits) - limit:]
                for w in extras:
                    out.append(
                        mybir.InstNoOp(
                            name=f"{ins.name}-wsp{n}", engine=ins.engine,
                            ins=[], outs=[],
                            sync_info=mybir.SyncInfo(on_wait=[w], on_update=[]),
                        )
                    )
                    n += 1
                ins.sync_info = mybir.SyncInfo(
                    on_wait=keep, on_update=list(si.on_update)
                )
            out.append(ins)
        bb.instructions = out
    return n


def _build():
    nc = bass.Bass()

    x_d = nc.dram_tensor("x", [BPC, CIN, T], F32R, kind="ExternalInput")
    tgt_d = nc.dram_tensor("tgt", [BPC * TDE], F32, kind="ExternalInput")
    dr_d = nc.dram_tensor("drf", [BPC, T], F32, kind="ExternalInput")
    w1_d = nc.dram_tensor("w1", [CIN, K1 * CH], F32R, kind="ExternalInput")
    w2_d = nc.dram_tensor("w2", [CH, K1 * CH], F32R, kind="ExternalInput")
    wlin_d = nc.dram_tensor("wlin", [CH], F32R, kind="ExternalInput")
    wlinS_d = nc.dram_tensor("wlinS", [1], F32R, kind="ExternalInput")
    blin_d = nc.dram_tensor("blin", [1], F32, kind="ExternalInput")
    b1_d = nc.dram_tensor("b1", [CH], F32, kind="ExternalInput")
    b2_d = nc.dram_tensor("b2", [CH], F32, kind="ExternalInput")
    bemb_d = nc.dram_tensor("bemb", [CH], F32, kind="ExternalInput")
    wemb_d = nc.dram_tensor("wemb", [KE, CH], F32R, kind="ExternalInput")
    iden_d = nc.dram_tensor("iden", [128, 128], F32, kind="ExternalInput")
    tri_d = nc.dram_tensor("tri", [128, 128], F32R, kind="ExternalInput")
    triu1_d = nc.dram_tensor("triu1", [128, 128], F32, kind="ExternalInput")
    jramp_d = nc.dram_tensor("jramp", [128, 512], F32, kind="ExternalInput")

    pred_d = nc.dram_tensor("pred", [BPC, T], F32, kind="ExternalOutput")
    avg_d = nc.dram_tensor("avg", [BPC, T], F32, kind="ExternalOutput")
    emb_d = nc.dram_tensor("emb", [BPC, CH, T], F32, kind="ExternalOutput")

    with tile.TileContext(nc) as tc:
        with (
            tc.tile_pool(name="const", bufs=1) as cpool,
            tc.tile_pool(name="work", bufs=2) as wpool,
            tc.tile_pool(name="seg", bufs=2) as spool,
            tc.tile_pool(name="rows", bufs=2) as rpool,
            tc.tile_pool(name="ps_conv", bufs=2, space="PSUM") as pconv,
            tc.tile_pool(name="ps_emb", bufs=2, space="PSUM") as pemb,
            tc.tile_pool(name="ps_reps", bufs=2, space="PSUM") as preps,
            tc.tile_pool(name="ps_stat", bufs=2, space="PSUM") as pstat,
        ):
            # ---- persistent constants / params ----
            iden = cpool.tile([128, 128], F32, tag="iden")
            tri = cpool.tile([128, 128], F32R, tag="tri")
            triu1 = cpool.tile([128, 128], F32, tag="triu1")
            jramp = cpool.tile([128, 512], F32, tag="jramp")
            w1sb = cpool.tile([128, 4 * K1 * CH], F32R, tag="w1")
            w2sb = cpool.tile([128, 2 * K1 * CH], F32R, tag="w2")
            wlin = cpool.tile([128, 2], F32R, tag="wlin")
            wlinS = cpool.tile([1, 1], F32R, tag="wlinS")
            blin = cpool.tile([1, 1], F32, tag="blin")
            b1c = cpool.tile([128, 2], F32, tag="b1c")
            b2c = cpool.tile([128, 2], F32, tag="b2c")
            bembc = cpool.tile([128, 2], F32, tag="bembc")
            wembT = cpool.tile([KE, CH], F32R, tag="wembT")
            epsc = cpool.tile([128, 1], F32, tag="epsc")
            zedge = cpool.tile([128, 8], F32, tag="zedge")

            nc.scalar.dma_start(iden[:], iden_d[:])
            nc.scalar.dma_start(tri[:], tri_d[:])
            nc.scalar.dma_start(triu1[:], triu1_d[:])
            nc.scalar.dma_start(jramp[:], jramp_d[:])
            for c in range(4):
                eng = [nc.sync, nc.scalar, nc.gpsimd, nc.sync][c]
                eng.dma_start(
                    w1sb[:, c * K1 * CH : (c + 1) * K1 * CH],
                    w1_d[c * 128 : (c + 1) * 128, :],
                )
            for c in range(2):
                eng = [nc.gpsimd, nc.scalar][c]
                eng.dma_start(
                    w2sb[:, c * K1 * CH : (c + 1) * K1 * CH],
                    w2_d[c * 128 : (c + 1) * 128, :],
                )
            for c in range(2):
                sl = slice(c * 128, (c + 1) * 128)
                nc.gpsimd.dma_start(
                    wlin[:, c : c + 1], wlin_d[sl].rearrange("(a b) -> a b", b=1)
                )
                nc.gpsimd.dma_start(
                    b1c[:, c : c + 1], b1_d[sl].rearrange("(a b) -> a b", b=1)
                )
                nc.gpsimd.dma_start(
                    b2c[:, c : c + 1], b2_d[sl].rearrange("(a b) -> a b", b=1)
                )
                nc.gpsimd.dma_start(
                    bembc[:, c : c + 1], bemb_d[sl].rearrange("(a b) -> a b", b=1)
                )
            nc.gpsimd.dma_start(blin[:], blin_d[:].rearrange("(a b) -> a b", b=1))
            nc.gpsimd.dma_start(wlinS[:], wlinS_d[:].rearrange("(a b) -> a b", b=1))
            nc.gpsimd.dma_start(wembT[:], wemb_d[:])
            nc.gpsimd.memset(epsc[:], EPS)
            nc.gpsimd.memset(zedge[:], 0.0)

            ones_row = tri[0:1, :]  # [1,128] all ones (f32r)
            ones_col = tri[:, 127:128]  # [128,1] all ones (f32r)

            def seg(b):
                _emit_segment(
                    nc, tc, b, tgt_d, dr_d, avg_d, emb_d,
                    iden, triu1, jramp, wembT, bembc,
                    wpool, spool, pemb, pstat,
                )

            def main(b):
                _emit_main(
                    nc, tc, b, x_d, pred_d,
                    iden, tri, ones_row, ones_col,
                    w1sb, w2sb, wlin, wlinS, blin, b1c, b2c, epsc, zedge,
                    wpool, rpool, pconv, preps, pstat, first=(b < 2),
                )

            seg(0); main(0); seg(1); main(1); seg(2); main(2); seg(3); main(3)

    _split_multi_waits(nc)
    return nc


def _emit_segment(
    nc, tc, b, tgt_d, dr_d, avg_d, emb_d,
    iden, triu1, jramp, wembT, bembc,
    wpool, spool, paux, pstat,
):
    """avg_energy_target for batch b + energy_emb conv.

    Row-major grouping: t = g*8 + i with group g on partition g.
    """
    # ends = inclusive cumsum of dr, via free-dim Hillis-Steele within each
    # 8-wide group plus a partition-dim exclusive prefix of group totals.
    dr_rm = spool.tile([128, 8], F32, tag="dr_rm")
    nc.sync.dma_start(dr_rm[:], dr_d[b, :].rearrange("(a b) -> a b", b=8))
    p1 = spool.tile([128, 8], F32, tag="p1")
    nc.vector.tensor_copy(p1[:, 0:1], dr_rm[:, 0:1])
    nc.vector.tensor_tensor(out=p1[:, 1:8], in0=dr_rm[:, 1:8], in1=dr_rm[:, 0:7], op=ALU.add)
    p2 = spool.tile([128, 8], F32, tag="p2")
    nc.vector.tensor_copy(p2[:, 0:2], p1[:, 0:2])
    nc.vector.tensor_tensor(out=p2[:, 2:8], in0=p1[:, 2:8], in1=p1[:, 0:6], op=ALU.add)
    ends = spool.tile([128, 8], F32, tag="ends")
    nc.vector.tensor_copy(ends[:, 0:4], p2[:, 0:4])
    nc.vector.tensor_tensor(out=ends[:, 4:8], in0=p2[:, 4:8], in1=p2[:, 0:4], op=ALU.add)

    gt_ps = pstat.tile([128, 512], F32, tag="stat")
    nc.tensor.matmul(gt_ps[:, 0:1], triu1[:], ends[:, 7:8], start=True, stop=True)
    offc = spool.tile([128, 1], F32, tag="offc")
    nc.scalar.activation(offc[:], gt_ps[:, 0:1], AF.Identity)
    nc.vector.tensor_scalar_add(ends[:], ends[:], offc[:, 0:1])
    starts = spool.tile([128, 8], F32, tag="starts")
    nc.vector.tensor_tensor(out=starts[:], in0=ends[:], in1=dr_rm[:], op=ALU.subtract)

    # window gather: one offset per partition (group), WIN consecutive values
    off_f = spool.tile([128, 1], F32, tag="off_f")
    nc.vector.tensor_scalar_add(off_f[:], starts[:, 0:1], float(b * TDE))
    off_i = spool.tile([128, 1], I32, tag="off_i")
    nc.vector.tensor_copy(off_i[:], off_f[:])
    W = spool.tile([128, WIN], F32, tag="W")
    nc.gpsimd.indirect_dma_start(
        out=W[:], out_offset=None,
        in_=tgt_d[:].rearrange("(a b) -> a b", b=1),
        in_offset=bass.IndirectOffsetOnAxis(ap=off_i[:], axis=0),
    )
    wnz = spool.tile([128, 2 * WIN], F32, tag="wnz")
    nc.vector.tensor_copy(wnz[:, 0:WIN], W[:])
    nc.vector.tensor_scalar(wnz[:, WIN : 2 * WIN], W[:], 0.0, None, ALU.not_equal)

    # per-partition relative end positions, then masked prefix reductions
    erel = spool.tile([128, 8], F32, tag="erel")
    nc.vector.tensor_scalar(
        erel[:], ends[:], starts[:, 0:1], None, ALU.subtract
    )
    # sene[:, 2i] = prefix sum_i, sene[:, 2i+1] = prefix count_i
    bigmask = spool.tile([128, 512], F32, tag="bigmask")
    erel_b = erel[:].rearrange("p (i o) -> p i o", o=1).to_broadcast((128, 8, 2 * WIN))
    nc.vector.tensor_tensor(
        out=bigmask[:].rearrange("p (i w) -> p i w", i=8),
        in0=jramp[:].rearrange("p (i w) -> p i w", i=8),
        in1=erel_b, op=ALU.is_lt,
    )
    bigprod = spool.tile([128, 512], F32, tag="bigprod")
    wnz_b = wnz[:].rearrange("p (o w) -> p o w", o=1).to_broadcast((128, 8, 2 * WIN))
    nc.vector.tensor_tensor(
        out=bigprod[:].rearrange("p (i w) -> p i w", i=8),
        in0=bigmask[:].rearrange("p (i w) -> p i w", i=8),
        in1=wnz_b, op=ALU.mult,
    )
    sene = spool.tile([128, 16], F32, tag="sene")
    nc.vector.tensor_reduce(
        out=sene[:].rearrange("p (i g) -> p i g", g=2),
        in_=bigprod[:].rearrange("p (i g w) -> p i g w", i=8, g=2),
        axis=mybir.AxisListType.X, op=ALU.add,
    )
    sene_r = sene[:].rearrange("p (i g) -> p i g", g=2)

    # segment sums / counts = adjacent differences of the prefix values
    sums = spool.tile([128, 8], F32, tag="sums")
    nc.vector.tensor_copy(sums[:, 0:1], sene[:, 0:1])
    nc.vector.tensor_tensor(
        out=sums[:, 1:8].rearrange("p (i o) -> p i o", o=1),
        in0=sene_r[:, 1:8, 0:1], in1=sene_r[:, 0:7, 0:1], op=ALU.subtract)
    nel = spool.tile([128, 8], F32, tag="nel")
    nc.vector.tensor_copy(nel[:, 0:1], sene[:, 1:2])
    nc.vector.tensor_tensor(
        out=nel[:, 1:8].rearrange("p (i o) -> p i o", o=1),
        in0=sene_r[:, 1:8, 1:2], in1=sene_r[:, 0:7, 1:2], op=ALU.subtract)

    dmax = spool.tile([128, 8], F32, tag="dmax")
    nc.vector.tensor_scalar_max(dmax[:], nel[:], 1.0)
    rec = spool.tile([128, 8], F32, tag="rec")
    nc.vector.reciprocal(rec[:], dmax[:])
    ind = spool.tile([128, 8], F32, tag="ind")
    nc.vector.tensor_scalar_min(ind[:], nel[:], 1.0)
    avg = spool.tile([128, 8], F32, tag="avgt")
    nc.vector.tensor_tensor(out=avg[:], in0=sums[:], in1=rec[:], op=ALU.mult)
    nc.vector.tensor_tensor(out=avg[:], in0=avg[:], in1=ind[:], op=ALU.mult)
    nc.gpsimd.dma_start(avg_d[b, :].rearrange("(a b) -> a b", b=8), avg[:])

    # --- energy_emb: K=3 matmul; rows built via SBUF shift DMAs ---
    row3s = spool.tile([KE, 1028], F32, tag="row3s")
    nc.gpsimd.memset(row3s[:], 0.0)
    nc.sync.dma_start(row3s[1:2, 0:1024], avg[:])
    nc.sync.dma_start(row3s[0:1, 1:1025], row3s[1:2, 0:1024])
    nc.sync.dma_start(row3s[2:3, 0:1023], row3s[1:2, 1:1024])
    row3 = spool.tile([KE, 1028], F32R, tag="row3")
    nc.vector.tensor_copy(row3[:], row3s[:])
    for cc in range(2):
        for tt0 in (0, 512):
            emb_ps = paux.tile([128, 512], F32, tag="embp")
            _mm(
                nc, emb_ps[:],
                wembT[:, cc * 128 : (cc + 1) * 128],
                row3[:, tt0 : tt0 + 512],
                start=True, stop=True,
            )
            emb_sb = wpool.tile([128, 512], F32, tag="emb_sb")
            nc.scalar.activation(
                emb_sb[:], emb_ps[:], AF.Identity, bias=bembc[:, cc : cc + 1]
            )
            nc.gpsimd.dma_start(
                emb_d[b, cc * 128 : (cc + 1) * 128, tt0 : tt0 + 512], emb_sb[:]
            )


def _emit_main(
    nc, tc, b, x_d, pred_d,
    iden, tri, ones_row, ones_col,
    w1sb, w2sb, wlin, wlinS, blin, b1c, b2c, epsc, zedge,
    wpool, rpool, pconv, preps, pstat, first=True,
):
    """VariancePredictor for batch b -> pred_d[b]."""
    # --- load + transpose x[b] to [CIN, T] padded ---
    xT = [wpool.tile([128, 1028], F32R, tag=f"xT{c}", name=f"xT{c}") for c in range(4)]
    if first:
        for c in range(4):
            nc.scalar.activation(xT[c][:, 0:2], zedge[:, 0:2], AF.Identity)
            nc.scalar.activation(xT[c][:, 1026:1028], zedge[:, 0:2], AF.Identity)
    for half in range(2):
        for c in range(4):
            nc.sync.dma_start(
                xT[c][:, 2 + half * 512 : 2 + (half + 1) * 512],
                x_d[b, c * 128 : (c + 1) * 128, half * 512 : (half + 1) * 512],
            )

    # --- conv1 + lrelu -> h1 (padded) ---
    h1 = [wpool.tile([128, 1028], F32R, tag=f"h1_{cc}", name=f"h1_{cc}") for cc in range(2)]
    if first:
        for cc in range(2):
            nc.scalar.activation(h1[cc][:, 0:2], zedge[:, 0:2], AF.Identity)
            nc.scalar.activation(h1[cc][:, 1026:1028], zedge[:, 0:2], AF.Identity)
    for tt0 in (0, 512):
        for cc in range(2):
            cps = pconv.tile([128, 512], F32, tag="conv")
            n = 0
            for ci in range(4):
                for k in range(K1):
                    _mm(
                        nc, cps[:],
                        w1sb[:, (ci * K1 + k) * CH + cc * 128 : (ci * K1 + k) * CH + cc * 128 + 128],
                        xT[ci][:, tt0 + k : tt0 + k + 512],
                        start=(n == 0), stop=(n == 19),
                    )
                    n += 1
            nc.scalar.activation(
                h1[cc][:, 2 + tt0 : 2 + tt0 + 512], cps[:], AF.Prelu,
                bias=b1c[:, cc : cc + 1], alpha=LEAKY,
            )

    # --- LN1 (normalize only; affine folded into w2/b2) ---
    _emit_ln(nc, b, 0, h1, 2, tri, ones_row, ones_col, epsc, wpool, rpool, preps, pstat)

    # --- conv2 + lrelu -> h2 ---
    h2 = [wpool.tile([128, 1024], F32R, tag=f"h2_{cc}", name=f"h2_{cc}") for cc in range(2)]
    for tt0 in (0, 512):
        for cc in range(2):
            cps = pconv.tile([128, 512], F32, tag="conv")
            n = 0
            for ci in range(2):
                for k in range(K1):
                    _mm(
                        nc, cps[:],
                        w2sb[:, (ci * K1 + k) * CH + cc * 128 : (ci * K1 + k) * CH + cc * 128 + 128],
                        h1[ci][:, tt0 + k : tt0 + k + 512],
                        start=(n == 0), stop=(n == 9),
                    )
                    n += 1
            nc.scalar.activation(
                h2[cc][:, tt0 : tt0 + 512], cps[:], AF.Prelu,
                bias=b2c[:, cc : cc + 1], alpha=LEAKY,
            )

    # --- LN2 (shift folded into pred matmul) ---
    wf2 = rpool.tile([1, 1024], F32R, tag="wf2")
    _emit_ln(nc, b, 1, h2, 0, tri, ones_row, ones_col, epsc, wpool, rpool, preps, pstat, w_out=wf2)

    # --- linear -> pred (+ S * w term) ---
    pred_sb = rpool.tile([1, 1024], F32, tag="pred_sb")
    for tt0 in (0, 512):
        pps = pstat.tile([1, 512], F32, tag="stat")
        _mm(nc, pps[:], wlin[:, 0:1], h2[0][:, tt0 : tt0 + 512], start=True, stop=False)
        _mm(nc, pps[:], wlin[:, 1:2], h2[1][:, tt0 : tt0 + 512], start=False, stop=False)
        _mm(nc, pps[:], wlinS[0:1, 0:1], wf2[0:1, tt0 : tt0 + 512], start=False, stop=True)
        nc.scalar.activation(
            pred_sb[:, tt0 : tt0 + 512], pps[:], AF.Identity, bias=blin[0:1, 0:1]
        )
    nc.gpsimd.dma_start(pred_d[b, :].rearrange("(a b) -> a b", a=1), pred_sb[:])


def _emit_ln(nc, b, which, h, pad, tri, ones_row, ones_col, epsc, wpool, rpool, preps, pstat, w_out=None):
    """In-place layernorm (normalize only) over channel dim of h ([CH,T] layout)."""
    inv_c = 1.0 / CH
    for tt0 in (0, 512):
        sl = slice(pad + tt0, pad + tt0 + 512)
        s1t = pstat.tile([1, 512], F32, tag="stat")
        s1 = s1t[:]
        _mm(nc, s1, ones_col, h[0][:, sl], start=True, stop=False)
        _mm(nc, s1, ones_col, h[1][:, sl], start=False, stop=True)
        negm = rpool.tile([1, 512], F32, tag="negm")
        nc.scalar.activation(negm[:], s1, AF.Identity, scale=-inv_c)
        s2t = pstat.tile([1, 512], F32, tag="stat")
        s2 = s2t[:]
        for cc in range(2):
            sq = wpool.tile([128, 512], F32R, tag="sq")
            nc.scalar.activation(sq[:], h[cc][:, sl], AF.Square)
            _mm(nc, s2, ones_col, sq[:], start=(cc == 0), stop=(cc == 1))
        msq = rpool.tile([1, 512], F32, tag="msq")
        nc.vector.tensor_tensor(out=msq[:], in0=negm[:], in1=negm[:], op=ALU.mult)
        s2c = rpool.tile([1, 512], F32, tag="s2c")
        nc.scalar.activation(s2c[:], s2, AF.Identity, scale=inv_c)
        var = rpool.tile([1, 512], F32, tag="var")
        nc.vector.tensor_tensor(out=var[:], in0=s2c[:], in1=msq[:], op=ALU.subtract)
        sd = rpool.tile([1, 512], F32, tag="sd")
        nc.scalar.activation(sd[:], var[:], AF.Sqrt, bias=epsc[0:1, 0:1])
        a_row = rpool.tile([1, 512], F32R, tag="a_row")
        with nc.allow_low_precision(reason="f32r operand for PE replicate"):
            nc.vector.reciprocal(a_row[:], sd[:])
        if w_out is not None:
            # shift folded downstream: h <- a * h only; w written to w_out row
            nc.vector.tensor_tensor(
                out=w_out[0:1, pad + tt0 : pad + tt0 + 512],
                in0=negm[:], in1=a_row[:].bitcast(F32), op=ALU.mult,
            )
            a_rep = preps.tile([128, 512], F32, tag="reps")
            _mm(nc, a_rep[:], ones_row, a_row[:], start=True, stop=True)
            for cc in range(2):
                nc.vector.tensor_tensor(
                    out=h[cc][:, sl], in0=h[cc][:, sl], in1=a_rep[:], op=ALU.mult
                )
        else:
            w_row = rpool.tile([1, 512], F32R, tag="w_row")
            nc.vector.tensor_tensor(out=w_row[:], in0=negm[:], in1=a_row[:].bitcast(F32), op=ALU.mult)
            a_rep = preps.tile([128, 512], F32, tag="reps")
            _mm(nc, a_rep[:], ones_row, a_row[:], start=True, stop=True)
            w_rep = preps.tile([128, 512], F32, tag="reps")
            _mm(nc, w_rep[:], ones_row, w_row[:], start=True, stop=True)
            for cc in range(2):
                nc.vector.tensor_tensor(
                    out=h[cc][:, sl], in0=h[cc][:, sl], in1=a_rep[:], op=ALU.mult
                )
                nc.vector.tensor_tensor(
                    out=h[cc][:, sl], in0=h[cc][:, sl], in1=w_rep[:], op=ALU.add
                )


@functools.lru_cache(maxsize=1)
def _get_nc():
    return _build()


def kernel(
    x, target, dr, mask,
    w_conv1, b_conv1, ln1_g, ln1_b,
    w_conv2, b_conv2, ln2_g, ln2_b,
    w_lin, b_lin, w_emb, b_emb,
):
    global LAST_EXEC_NS, LAST_RESULTS

    x = np.ascontiguousarray(np.asarray(x, np.float32).transpose(0, 2, 1))
    tgt = np.ascontiguousarray(np.asarray(target, np.float32).reshape(B, TDE))
    drf = np.ascontiguousarray(np.asarray(dr).astype(np.float32))
    mask_np = np.asarray(mask)

    w1h = np.asarray(w_conv1, np.float32)  # [CH, CIN, K]
    g1 = np.asarray(ln1_g, np.float32)
    b1ln = np.asarray(ln1_b, np.float32)
    w2h = np.asarray(w_conv2, np.float32) * g1[None, :, None]
    b2_eff = np.asarray(b_conv2, np.float32) + np.einsum(
        "oik,i->o", np.asarray(w_conv2, np.float32), b1ln
    )
    g2 = np.asarray(ln2_g, np.float32)
    b2ln = np.asarray(ln2_b, np.float32)
    wlin_h = np.asarray(w_lin, np.float32)[:, 0]
    wlin_eff = wlin_h * g2
    blin_eff = (np.asarray(b_lin, np.float32) + wlin_h @ b2ln).reshape(1)
    wlinS_v = np.asarray([wlin_eff.sum()], np.float32)

    w1 = np.ascontiguousarray(w1h.transpose(1, 2, 0).reshape(CIN, K1 * CH))
    w2 = np.ascontiguousarray(w2h.transpose(1, 2, 0).reshape(CH, K1 * CH))
    wembT = np.ascontiguousarray(np.asarray(w_emb, np.float32)[:, 0, :].T)  # [KE, CH]
    b1c = np.asarray(b_conv1, np.float32)
    bemb = np.asarray(b_emb, np.float32)

    iden = np.eye(128, dtype=np.float32)
    tri = np.triu(np.ones((128, 128), np.float32))
    triu1 = np.triu(np.ones((128, 128), np.float32), 1)
    jr = np.tile(np.concatenate([np.arange(WIN), np.arange(WIN)]), 8).astype(np.float32)
    jramp = np.broadcast_to(jr[None, :], (128, 512)).copy()

    shared = {
        "w1": w1, "w2": w2, "wlin": np.ascontiguousarray(wlin_eff),
        "blin": np.ascontiguousarray(blin_eff),
        "wlinS": wlinS_v,
        "b1": np.ascontiguousarray(b1c), "b2": np.ascontiguousarray(b2_eff),
        "bemb": np.ascontiguousarray(bemb), "wemb": wembT,
        "iden": iden, "tri": tri, "triu1": triu1, "jramp": jramp,
    }
    in_maps = []
    for c in range(NCORES):
        sl = slice(c * BPC, (c + 1) * BPC)
        in_maps.append(
            {
                "x": np.ascontiguousarray(x[sl]),
                "tgt": np.ascontiguousarray(tgt[sl]).reshape(-1),
                "drf": np.ascontiguousarray(drf[sl]),
                **shared,
            }
        )

    nc = _get_nc()
    res = run_bass_kernel_spmd(nc, in_maps, list(range(NCORES)), trace=PROFILE)
    LAST_EXEC_NS = res.exec_time_ns
    LAST_RESULTS = res

    pred = np.concatenate([r["pred"] for r in res.results], axis=0)
    avg = np.concatenate([r["avg"] for r in res.results], axis=0)
    emb = np.concatenate([r["emb"] for r in res.results], axis=0)

    pred = np.where(mask_np, np.float32(0.0), pred).astype(np.float32)
    return pred, avg.reshape(B, 1, T), emb
